# revision 2
# baseline (speedup 1.0000x reference)
"""Trainium2 Bass kernel for nn_RelationDecoder — collective (b-shard) design.

Each core reads only a 32-wide b-slice of T_he/T_te (8.4 MB instead of
67 MB), computes hep/tep for ALL k on that slice, then ONE AllToAll per T
exchanges [k-shard x b-slice] blocks so each core ends with its k-shard over
the FULL b range.  Phases 5/6 (T_cls contraction + final pred) then run
k-sharded as in the replicated baseline.

Emission order is engine-queue aware: all LSTM matmuls first (PE), gathers
ride gpsimd (including the span max-pool) so they never head-of-line-block
the PE/DVE queues, and each pool's PE transpose is emitted only right before
its first PE consumer.
"""

import os
import tempfile

import numpy as np
import ml_dtypes

import concourse.bass as bass
import concourse.mybir as mybir
import concourse.tile as tile
from concourse import bacc
from concourse.bass import IndirectOffsetOnAxis
from concourse.masks import make_identity

SEQ, D, HID, C = 512, 768, 128, 5
NENT, SPAN = 128, 16
H2 = 2 * HID            # 256
NCORES = 8
KSH = NENT // NCORES    # 16 k per core after the a2a
BSH = H2 // NCORES      # 32 b per core before the a2a
P = 128
DAUG = D + P            # 768 + bias row + pad = 896
NDCH = DAUG // P
GATES = 3 * HID         # i, g, o gates (f dead: c0=0)

BF = mybir.dt.bfloat16
F32 = mybir.dt.float32
I32 = mybir.dt.int32
AF = mybir.ActivationFunctionType

_NC_CACHE = {}
DEBUG = bool(int(os.environ.get("KERNEL_DEBUG", "0")))


class _CopyVia:
    def __init__(self, eng, is_vec):
        self.eng, self.is_vec = eng, is_vec

    def tensor_copy(self, out, in_):
        if self.is_vec:
            self.eng.tensor_copy(out=out, in_=in_)
        else:
            self.eng.copy(out, in_)


def _copy_engine(nc, idx):
    return _CopyVia(nc.vector, True) if idx % 2 == 0 else _CopyVia(nc.scalar, False)


def build_nc():
    if "nc" in _NC_CACHE:
        return _NC_CACHE["nc"]

    nc = bacc.Bacc(
        "TRN2",
        target_bir_lowering=False,
        debug=False,
        enable_asserts=False,
        num_devices=NCORES,
    )

    # ---- I/O -------------------------------------------------------------
    enc_t = nc.dram_tensor("enc_t", [DAUG, SEQ], BF, kind="ExternalInput")
    wts = nc.dram_tensor("wts", [6, DAUG, GATES], BF, kind="ExternalInput")
    t_te_sl = nc.dram_tensor("t_te_sl", [H2, BSH, H2], BF, kind="ExternalInput")
    t_he_sl = nc.dram_tensor("t_he_sl", [H2, BSH, H2], BF, kind="ExternalInput")
    t_cls = nc.dram_tensor("t_cls", [H2, C, H2], BF, kind="ExternalInput")  # [b,m,c]
    hold_idx = nc.dram_tensor("hold_idx", [NENT, SPAN], I32, kind="ExternalInput")
    targ_idx = nc.dram_tensor("targ_idx", [NENT, SPAN], I32, kind="ExternalInput")
    exp_idx = nc.dram_tensor("exp_idx", [NENT, SPAN], I32, kind="ExternalInput")
    pred_out = nc.dram_tensor("pred_out", [KSH, NENT, C, NENT], BF,
                              kind="ExternalOutput")
    dbg = {}
    if DEBUG:
        dbg["pool"] = nc.dram_tensor("dbg_pool", [3, NENT, H2], BF,
                                     kind="ExternalOutput")
        dbg["u"] = nc.dram_tensor("dbg_u", [P, BSH, H2], BF, kind="ExternalOutput")
        dbg["ep"] = nc.dram_tensor("dbg_ep", [P, BSH, P], BF, kind="ExternalOutput")
        dbg["tepT"] = nc.dram_tensor("dbg_tepT", [P, 2, KSH, P], BF,
                                     kind="ExternalOutput")
        dbg["hepT"] = nc.dram_tensor("dbg_hepT", [P, 2, KSH, P], BF,
                                     kind="ExternalOutput")

    h_tab = {n: nc.dram_tensor(f"h_tab_{n}", [SEQ, H2], BF, kind="Internal")
             for n in ("ee", "te", "he")}
    # block kd = [b, kw, i]: the write side eats the small (256 B) transpose
    # descriptors off the critical path; the readback gets 4 KB descriptors.
    a2a_in = {n: nc.dram_tensor(f"a2a_in_{n}", [NCORES, BSH, KSH, P], BF,
                                kind="Internal") for n in ("te", "he")}
    a2a_out = {n: nc.dram_tensor(f"a2a_out_{n}", [NCORES, BSH, KSH, P], BF,
                                 kind="Internal") for n in ("te", "he")}
    RG = [list(range(NCORES))]

    from contextlib import ExitStack
    with tile.TileContext(nc) as tcx, ExitStack() as stk:
        const = stk.enter_context(tcx.tile_pool(name="const", bufs=1))
        work = stk.enter_context(tcx.tile_pool(name="work", bufs=2))
        hpool = stk.enter_context(tcx.tile_pool(name="hpool", bufs=8))
        tslp = stk.enter_context(tcx.tile_pool(name="tsl", bufs=16))
        gpool = stk.enter_context(tcx.tile_pool(name="gpool", bufs=2))
        big = stk.enter_context(tcx.tile_pool(name="big", bufs=1))
        u2p = stk.enter_context(tcx.tile_pool(name="u2p", bufs=3))
        tclsp = stk.enter_context(tcx.tile_pool(name="tclsp", bufs=2))

        # ---- persistent SBUF loads --------------------------------------
        enc_sb = const.tile([P, NDCH, SEQ], BF)
        nc.sync.dma_start(enc_sb[:], enc_t.ap().rearrange("(n p) s -> p n s", p=P))
        wt_sb = const.tile([P, NDCH, 6, GATES], BF)
        for w in (4, 5, 2, 3, 0, 1):        # ee weights first: LSTM-ee gates all
            nc.sync.dma_start(wt_sb[:, :, w, :],
                              wts.ap()[w].rearrange("(n p) g -> p n g", p=P))
        idx_sb = {}
        for n, t in (("ee", exp_idx), ("te", targ_idx), ("he", hold_idx)):
            idx_sb[n] = const.tile([NENT, SPAN], I32, name=f"idx_{n}")
            nc.sync.dma_start(idx_sb[n][:], t.ap())
        tcls_sb = const.tile([P, 2, C, H2], BF)
        nc.sync.dma_start(tcls_sb[:], t_cls.ap().rearrange("(bc p) m c -> p bc m c", p=P))
        ident = const.tile([P, P], BF)

        # T-slice prefetch: all 16 tiles are resident (bufs=16).  The DMA
        # rings are FIFO, so each pool's h-table writes must hit the rings
        # BEFORE the next 4.2 MB T-slice batch: emission is interleaved with
        # the LSTM pools below (te slices after the ee h-writes, he slices
        # after the te h-writes).
        tsl_tiles = {"te": {}, "he": {}}

        def emit_tsl(nm, t_dram):
            for bg in range(4):
                for cc in range(2):
                    rt = tslp.tile([P, 8, H2], BF, tag="tsl")
                    nc.sync.dma_start(
                        rt[:], t_dram.ap()[cc * P:(cc + 1) * P,
                                           bg * 8:(bg + 1) * 8, :])
                    tsl_tiles[nm][(bg, cc)] = rt

        with tcx.tile_pool(name="ps_ls", bufs=2, space="PSUM") as ps_ls, \
             tcx.tile_pool(name="ps_tr", bufs=2, space="PSUM") as ps_tr, \
             tcx.tile_pool(name="ps_a", bufs=2, space="PSUM") as ps_a, \
             tcx.tile_pool(name="ps_b", bufs=2, space="PSUM") as ps_b:

            # ---- phase 1: LSTM h tables, all 3 pools (PE stays hot) ------
            WIDX = {"he": 0, "te": 2, "ee": 4}
            for pool_n in ("ee", "te", "he"):
                for tb in range(SEQ // P):
                    h_tile = hpool.tile([P, H2], BF, tag="h_tile")
                    for d in range(2):
                        ps_g = ps_ls.tile([P, GATES], F32, tag="ps_g")
                        for dc in range(NDCH):
                            nc.tensor.matmul(
                                ps_g[:],
                                lhsT=enc_sb[:, dc, tb * P:(tb + 1) * P],
                                rhs=wt_sb[:, dc, WIDX[pool_n] + d, :],
                                start=(dc == 0),
                                stop=(dc == NDCH - 1),
                            )
                        ti = work.tile([P, HID], F32, tag="ti")
                        nc.scalar.activation(ti[:], ps_g[:, 0:HID], AF.Sigmoid)
                        tg = work.tile([P, HID], F32, tag="tg")
                        nc.scalar.activation(tg[:], ps_g[:, HID:2 * HID], AF.Tanh)
                        cc_ = work.tile([P, HID], F32, tag="cc_")
                        nc.vector.tensor_mul(cc_[:], ti[:], tg[:])
                        tc_ = work.tile([P, HID], F32, tag="tc_")
                        nc.scalar.activation(tc_[:], cc_[:], AF.Tanh)
                        to = work.tile([P, HID], F32, tag="to")
                        nc.scalar.activation(to[:], ps_g[:, 2 * HID:3 * HID], AF.Sigmoid)
                        nc.vector.tensor_mul(h_tile[:, d * HID:(d + 1) * HID],
                                             to[:], tc_[:])
                    nc.sync.dma_start(h_tab[pool_n][tb * P:(tb + 1) * P, :], h_tile[:])
                if pool_n == "ee":
                    emit_tsl("te", t_te_sl)
                elif pool_n == "te":
                    emit_tsl("he", t_he_sl)

            # ---- phase 2: span gathers, all on gpsimd --------------------
            g_tiles = {}
            for pn in ("ee", "te", "he"):
                g_t = gpool.tile([NENT, SPAN, H2], BF, tag="gath", name=f"g_{pn}")
                for l in range(SPAN):
                    nc.gpsimd.indirect_dma_start(
                        out=g_t[:, l, :],
                        out_offset=None,
                        in_=h_tab[pn].ap(),
                        in_offset=IndirectOffsetOnAxis(ap=idx_sb[pn][:, l:l + 1],
                                                       axis=0),
                    )
                g_tiles[pn] = g_t

            # identity for PE transposes — emitted after the gathers so its
            # gpsimd memset/affine_select doesn't delay the gather chain
            make_identity(nc, ident[:])

            # max-pool (DVE) and feature-chunk transpose [P, 2, 128] are
            # emitted separately: the reduce goes into the DVE queue as early
            # as its gather allows, the PE transpose only right before its
            # first PE consumer (in-order engine queues: a too-early emission
            # head-of-line-blocks the whole engine).
            DPI = {"ee": 0, "te": 1, "he": 2}
            pooled = {}

            def pool_reduce(pn):
                # two half-reduces + a max: the first half starts as soon as
                # gathers 0-7 land instead of waiting for all 16
                pooled[pn] = const.tile([NENT, H2], BF, name=f"pool_{pn}")
                ha = work.tile([NENT, H2], BF, tag="pool_ha")
                nc.vector.reduce_max(
                    out=ha[:],
                    in_=g_tiles[pn][:, 0:SPAN // 2, :].rearrange("p l f -> p f l"),
                    axis=mybir.AxisListType.X,
                )
                hb = work.tile([NENT, H2], BF, tag="pool_hb")
                nc.vector.reduce_max(
                    out=hb[:],
                    in_=g_tiles[pn][:, SPAN // 2:, :].rearrange("p l f -> p f l"),
                    axis=mybir.AxisListType.X,
                )
                nc.vector.tensor_tensor(out=pooled[pn][:], in0=ha[:], in1=hb[:],
                                        op=mybir.AluOpType.max)
                if DEBUG:
                    nc.sync.dma_start(dbg["pool"].ap()[DPI[pn]], pooled[pn][:])

            def pool_transpose(pn):
                dst = const.tile([P, 2, NENT], BF, name=f"T_{pn}")
                for ch in range(2):
                    ps_t = ps_tr.tile([P, P], BF, tag="ps_trt")
                    nc.tensor.transpose(
                        ps_t[:, :NENT],
                        pooled[pn][:, ch * P:(ch + 1) * P],
                        ident[:NENT, :NENT],
                    )
                    _copy_engine(nc, ch).tensor_copy(out=dst[:, ch, :],
                                                     in_=ps_t[:, :NENT])
                return dst

            pool_reduce("ee")
            eeT = pool_transpose("ee")

            # ---- phase 3 per T: stage A, transposes, stage B, epT, a2a ----
            ci = 0
            for name, t_dram in (("te", t_te_sl), ("he", t_he_sl)):
                # stage A: U[k,(b,a)] = sum_c Ee[k,c] T[c,b,a], streamed in
                # 512-col tiles; each tile's 4 PE transposes (-> UT[a,ac,b,k])
                # are emitted one iteration later so they never stall the PE
                # behind their own PSUM->SBUF copy.
                ut_sb = big.tile([P, 2, BSH, P], BF, tag="ut_sb")

                def emit_utr(pend):
                    sg, u2 = pend
                    for db in range(2):
                        for ac in range(2):
                            ps_t = ps_tr.tile([P, P], BF, tag="ps_trt")
                            nc.tensor.transpose(
                                ps_t[:], u2[:, db, ac * P:(ac + 1) * P], ident[:])
                            _copy_engine(nc, emit_utr.ci).tensor_copy(
                                out=ut_sb[:, ac, sg * 2 + db, :], in_=ps_t[:])
                            emit_utr.ci += 1

                emit_utr.ci = ci
                pend = None
                for bg in range(4):         # groups of 8 b
                    r = [tsl_tiles[name][(bg, cc)][:].rearrange("p b a -> p (b a)")
                         for cc in range(2)]
                    for s in range(4):
                        ps_u = ps_a.tile([P, 512], F32, tag="ps_u")
                        for cc in range(2):
                            nc.tensor.matmul(
                                ps_u[:],
                                lhsT=eeT[:, cc, :],
                                rhs=r[cc][:, s * 512:(s + 1) * 512],
                                start=(cc == 0),
                                stop=(cc == 1),
                            )
                        u2 = u2p.tile([P, 2, H2], BF, tag="u2")
                        _copy_engine(nc, ci).tensor_copy(
                            out=u2[:].rearrange("p b a -> p (b a)"), in_=ps_u[:])
                        ci += 1
                        if DEBUG and name == "te":
                            sg_ = bg * 4 + s
                            nc.sync.dma_start(
                                dbg["u"].ap()[:, sg_ * 2:sg_ * 2 + 2, :], u2[:])
                        if pend is not None:
                            emit_utr(pend)
                        pend = (bg * 4 + s, u2)
                emit_utr(pend)
                ci = emit_utr.ci

                # pool reduce + entity transpose right before their PE use
                # (the reduce is gather-bound; emitted earlier it would
                # head-of-line-block stage A's copies in the DVE queue)
                pool_reduce(name)
                sT = pool_transpose(name)

                # stage B: ep[i,(b,k)] = sum_a He[i,a] UT[a,(b,k)]; the per-b
                # ep transposes (-> epl[k-part, b, i]) pipeline one step back
                epl = big.tile([P, BSH, P], BF, tag="epl")
                ut_flat = [ut_sb[:, ac, :, :].rearrange("p b k -> p (b k)")
                           for ac in range(2)]

                def emit_eptr(pend):
                    bg, ep2 = pend
                    for db in range(4):
                        ps_t = ps_tr.tile([P, P], BF, tag="ps_trt")
                        nc.tensor.transpose(ps_t[:], ep2[:, db, :], ident[:])
                        _copy_engine(nc, emit_eptr.ci).tensor_copy(
                            out=epl[:, bg * 4 + db, :], in_=ps_t[:])
                        emit_eptr.ci += 1

                emit_eptr.ci = ci
                pend = None
                for bg in range(8):         # groups of 4 b = 512 cols
                    ps_h = ps_b.tile([P, 512], F32, tag="ps_h")
                    for ac in range(2):
                        nc.tensor.matmul(
                            ps_h[:],
                            lhsT=sT[:, ac, :],
                            rhs=ut_flat[ac][:, bg * 512:(bg + 1) * 512],
                            start=(ac == 0),
                            stop=(ac == 1),
                        )
                    ep2 = u2p.tile([P, 4, P], BF, tag="ep2")
                    _copy_engine(nc, ci).tensor_copy(
                        out=ep2[:].rearrange("p b k -> p (b k)"), in_=ps_h[:])
                    ci += 1
                    if DEBUG and name == "te":
                        nc.sync.dma_start(
                            dbg["ep"].ap()[:, bg * 4:(bg + 1) * 4, :], ep2[:])
                    if pend is not None:
                        emit_eptr(pend)
                    pend = (bg, ep2)
                emit_eptr(pend)
                ci = emit_eptr.ci

                for kd in range(NCORES):
                    nc.sync.dma_start(
                        a2a_in[name].ap()[kd].rearrange("b kw i -> kw b i"),
                        epl[kd * KSH:(kd + 1) * KSH, :, :])
                nc.gpsimd.collective_compute(
                    "AllToAll", mybir.AluOpType.bypass, replica_groups=RG,
                    ins=[a2a_in[name].ap().opt()],
                    outs=[a2a_out[name].ap().opt()],
                )

        # ---- phase 4: readback  epT_full[(src,b32)-part, bc, kw, i] -------
        def readback(name):
            t = const.tile([P, 2, KSH, P], BF, name=f"{name}T_full")
            for s in range(NCORES):
                bc, s4 = divmod(s, 4)
                nc.sync.dma_start(
                    t[s4 * BSH:(s4 + 1) * BSH, bc, :, :],
                    a2a_out[name].ap()[s])
            return t

        tepT = readback("te")
        hepT = readback("he")
        if DEBUG:
            nc.sync.dma_start(dbg["tepT"].ap(), tepT[:])
            nc.sync.dma_start(dbg["hepT"].ap(), hepT[:])

        # ---- phase 5+6 per k-quad ----------------------------------------
        with tcx.tile_pool(name="ps5", bufs=2, space="PSUM") as ps5, \
             tcx.tile_pool(name="ps6", bufs=2, space="PSUM") as ps6:
            ci6 = 0
            tcls_tiles = {}

            def emit_ph5(kq):
                nonlocal ci6
                tclsT = tclsp.tile([P, 2, 4, C, P], BF, tag="tclsT")
                for m in range(C):
                    for cc in range(2):
                        ps_t5 = ps5.tile([P, 512], F32, tag="ps_t5")
                        for bc in range(2):
                            nc.tensor.matmul(
                                ps_t5[:],
                                lhsT=tcls_sb[:, bc, m, cc * P:(cc + 1) * P],
                                rhs=tepT[:, bc, 4 * kq:4 * (kq + 1), :]
                                    .rearrange("p kk j -> p (kk j)"),
                                start=(bc == 0),
                                stop=(bc == 1),
                            )
                        _copy_engine(nc, ci6).tensor_copy(
                            out=tclsT[:, cc, :, m, :],
                            in_=ps_t5[:].rearrange("p (kk j) -> p kk j", kk=4))
                        ci6 += 1
                tcls_tiles[kq] = tclsT

            def emit_ph6(kq):
                nonlocal ci6
                tclsT = tcls_tiles.pop(kq)
                for kk in range(4):
                    k = kq * 4 + kk
                    ps_p = ps6.tile([P, C, P], F32, tag="ps_p")
                    ps_flat = ps_p[:].rearrange("p m j -> p (m j)")
                    for cc in range(2):
                        rhs_flat = tclsT[:, cc, kk, :, :].rearrange("p m j -> p (m j)")
                        nc.tensor.matmul(
                            ps_flat[:, 0:512],
                            lhsT=hepT[:, cc, k, :],
                            rhs=rhs_flat[:, 0:512],
                            start=(cc == 0),
                            stop=(cc == 1),
                        )
                        nc.tensor.matmul(
                            ps_flat[:, 512:640],
                            lhsT=hepT[:, cc, k, :],
                            rhs=rhs_flat[:, 512:640],
                            start=(cc == 0),
                            stop=(cc == 1),
                        )
                    pred_sb = work.tile([P, C, P], BF, tag="pred_sb")
                    _copy_engine(nc, ci6).tensor_copy(out=pred_sb[:], in_=ps_p[:])
                    ci6 += 1
                    nc.sync.dma_start(pred_out.ap()[k], pred_sb[:])

            # staggered by one k-quad (tclsp holds 2 tiles): ph5 needs only
            # tepT, so ph5(kq+1) fills the PE while ph6 waits the he readback
            emit_ph5(0)
            emit_ph5(1)
            emit_ph6(0)
            emit_ph5(2)
            emit_ph6(1)
            emit_ph5(3)
            emit_ph6(2)
            emit_ph6(3)

    nc.compile()
    _NC_CACHE["nc"] = nc
    return nc


def prep_inputs(inputs):
    bf16 = ml_dtypes.bfloat16
    enc = np.asarray(inputs["encoder_output"], np.float32)[0]
    enc_aug = np.zeros((DAUG, SEQ), np.float32)
    enc_aug[:D] = enc.T
    enc_aug[D] = 1.0

    wts = np.zeros((6, DAUG, GATES), np.float32)
    for wi, (wn, bn) in enumerate([("Wh_f", "bh_f"), ("Wh_b", "bh_b"),
                                   ("Wt_f", "bt_f"), ("Wt_b", "bt_b"),
                                   ("We_f", "be_f"), ("We_b", "be_b")]):
        W = np.asarray(inputs[wn], np.float32)
        b = np.asarray(inputs[bn], np.float32)
        keep = np.r_[0:HID, 2 * HID:4 * HID]
        wts[wi, :D] = W[keep].T
        wts[wi, D] = b[keep]

    # [a, b, c] -> [c, b, a]
    t_he = np.asarray(inputs["T_he"], np.float32).transpose(2, 1, 0).astype(bf16)
    t_te = np.asarray(inputs["T_te"], np.float32).transpose(2, 1, 0).astype(bf16)

    shared = {
        "enc_t": enc_aug.astype(bf16),
        "wts": wts.astype(bf16),
        "t_cls": np.ascontiguousarray(
            np.asarray(inputs["T_cls"], np.float32)).astype(bf16),
        "hold_idx": np.ascontiguousarray(np.asarray(inputs["holder_idxs"], np.int32)),
        "targ_idx": np.ascontiguousarray(np.asarray(inputs["target_idxs"], np.int32)),
        "exp_idx": np.ascontiguousarray(np.asarray(inputs["exp_idxs"], np.int32)),
    }
    in_maps = []
    for c in range(NCORES):
        m = dict(shared)
        m["t_he_sl"] = np.ascontiguousarray(t_he[:, c * BSH:(c + 1) * BSH, :])
        m["t_te_sl"] = np.ascontiguousarray(t_te[:, c * BSH:(c + 1) * BSH, :])
        in_maps.append(m)
    return in_maps


def kernel(**inputs) -> np.ndarray:
    nc = build_nc()
    in_maps = prep_inputs(inputs)
    trace = bool(int(os.environ.get("KERNEL_TRACE", "0")))
    kwargs = {}
    if trace:
        kwargs = dict(trace=True, tmpdir=tempfile.mkdtemp(prefix="rd2_neff_"))
        if bool(int(os.environ.get("KERNEL_TRACE_ALL", "0"))):
            kwargs["trace_cores"] = list(range(NCORES))
    res = bass_run(nc, in_maps, **kwargs)
    outs = [np.asarray(r["pred_out"], np.float32) for r in res.results]
    full = np.concatenate([o.transpose(1, 3, 0, 2) for o in outs], axis=2)
    kernel.last_result = res
    return np.ascontiguousarray(full)                   # [i, j, k, m] fp32


def bass_run(nc, in_maps, **kwargs):
    from concourse.bass_utils import run_bass_kernel_spmd
    return run_bass_kernel_spmd(nc, in_maps, core_ids=list(range(NCORES)), **kwargs)


if __name__ == "__main__":
    import reference
    inputs = reference.setup_inputs()
    out = kernel(**{k: np.asarray(v) for k, v in inputs.items()})
    print("kernel output", out.shape, out.dtype)
